# revision 10
# baseline (speedup 1.0000x reference)
"""PositionAttention kernel for 8 TRN2 NeuronCores.

Sharding: the 32 (batch*head) pairs are split 4-per-core. Core c handles
batch b = c//2, heads H0..H0+3 where H0 = 4*(c%2). Each core computes its
4 heads' attention plus its slice of the in/out projections, producing a
partial [1024, 512] output for its batch; the host sums the two partials
per batch (tensor-parallel unshard) and adds out_b.

Device math (per core), all in a transposed score layout S'[k, q]:
  qT/kT = (W x^T)           [256, 1024]  (scale folded into Wq on host)
  v     = (x W^T)           [1024, 256] + ones column per head (denom trick)
  S'    = K_h @ Q_h^T + kernel[k, q]    (bias added via identity-matmul
                                         accumulate into PSUM; mask applied
                                         to the bias tile via copy_predicated)
  E     = exp(S')           (ACT engine)
  O     = E^T @ [v_h | 1]   [q, 65] -> col 64 is the softmax denominator
  Onorm = O[:, :64] * (1 / O[:, 64])    (per-partition scalar multiply)
  out   = (Onorm stacked over heads) @ ow_slice   (after PE transpose)
"""

import numpy as np

B, L, HD, H, D = 4, 1024, 512, 8, 64
NCORES = 8
SCALE = 1.0 / 8.0
NEG = -30000.0

_PROGRAM = None


def _build_program():
    from contextlib import ExitStack

    import concourse.bass as bass
    import concourse.mybir as mybir
    import concourse.tile as tile
    from concourse import bacc
    from concourse.masks import make_identity

    f32 = mybir.dt.float32
    u8 = mybir.dt.uint8
    EXP = mybir.ActivationFunctionType.Exp

    nc = bacc.Bacc(None, target_bir_lowering=False)

    xq_d = nc.declare_dram_parameter("xq", [512, 1024], f32, isOutput=False)
    xk_d = nc.declare_dram_parameter("xk", [512, 1024], f32, isOutput=False)
    xv_d = nc.declare_dram_parameter("xv", [512, 1024], f32, isOutput=False)
    wq_d = nc.declare_dram_parameter("wq", [512, 256], f32, isOutput=False)
    wk_d = nc.declare_dram_parameter("wk", [512, 256], f32, isOutput=False)
    wv_d = nc.declare_dram_parameter("wv", [512, 256], f32, isOutput=False)
    bq_d = nc.declare_dram_parameter("bq", [128, 2], f32, isOutput=False)
    bk_d = nc.declare_dram_parameter("bk", [128, 2], f32, isOutput=False)
    bv_d = nc.declare_dram_parameter("bv", [1, 256], f32, isOutput=False)
    ow_d = nc.declare_dram_parameter("ow", [256, 512], f32, isOutput=False)
    kern_d = nc.declare_dram_parameter("kern", [4, 1024, 1024], f32, isOutput=False)
    mask_d = nc.declare_dram_parameter("maskT", [4, 1024, 1024], u8, isOutput=False)
    out_d = nc.declare_dram_parameter("partial", [1024, 512], f32, isOutput=True)

    with tile.TileContext(nc) as tc, ExitStack() as top:
        const = top.enter_context(tc.tile_pool(name="const", bufs=1))
        big = top.enter_context(tc.tile_pool(name="big", bufs=1))

        ident = const.tile([128, 128], f32, tag="ident")
        make_identity(nc, ident)
        neginf = const.tile([128, 512], f32, tag="neginf")
        nc.vector.memset(neginf, NEG)

        wq_sb = const.tile([128, 1024], f32, tag="wq")
        wk_sb = const.tile([128, 1024], f32, tag="wk")
        wv_sb = const.tile([128, 1024], f32, tag="wv")
        ow_sb = const.tile([128, 1024], f32, tag="ow")
        bq_sb = const.tile([128, 2], f32, tag="bq")
        bk_sb = const.tile([128, 2], f32, tag="bk")
        bv_sb = const.tile([128, 256], f32, tag="bv")
        nc.sync.dma_start(out=wq_sb.rearrange("p (c m) -> p c m", c=4),
                          in_=wq_d[:].rearrange("(c p) m -> p c m", p=128))
        nc.sync.dma_start(out=wk_sb.rearrange("p (c m) -> p c m", c=4),
                          in_=wk_d[:].rearrange("(c p) m -> p c m", p=128))
        nc.sync.dma_start(out=wv_sb.rearrange("p (c m) -> p c m", c=4),
                          in_=wv_d[:].rearrange("(c p) m -> p c m", p=128))
        nc.sync.dma_start(out=ow_sb.rearrange("p (c n) -> p c n", c=2),
                          in_=ow_d[:].rearrange("(c p) n -> p c n", p=128))
        nc.sync.dma_start(out=bq_sb, in_=bq_d[:])
        nc.sync.dma_start(out=bk_sb, in_=bk_d[:])
        bv_ap = bv_d[:]
        bv_bcast = bass.AP(tensor=bv_ap.tensor, offset=bv_ap.offset,
                           ap=[[0, 128]] + list(bv_ap.ap)[1:])
        nc.sync.dma_start(out=bv_sb, in_=bv_bcast)

        # persistent SBUF: qT/kT [256, 1024] as 2 m-chunks side by side;
        # v as 8 k-chunks of [128, 4*65] (64 data cols + ones col per head);
        # OnSb [q, 8 qq-chunks * 256]; OnT [256, 1024] as 2 d-chunks.
        qT_sb = big.tile([128, 2048], f32, tag="qT")
        kT_sb = big.tile([128, 2048], f32, tag="kT")
        v_sb = big.tile([128, 8 * 260], f32, tag="v")
        on_sb = big.tile([128, 2048], f32, tag="on")
        ont_sb = big.tile([128, 2048], f32, tag="ont")

        # ---- stage A/B: projections ----
        with tc.tile_pool(name="xin", bufs=1) as xin, \
             tc.tile_pool(name="psA", bufs=2, space="PSUM") as psA, \
             tc.tile_pool(name="psTch", bufs=1, space="PSUM") as psTch:
            # Walrus codegen allows only ONE semaphore wait on a Matmult
            # (the LDWEIGHTS half has a single wait slot). Tiny "touch"
            # matmuls absorb one producer semaphore each so every real
            # matmul needs at most one fresh wait.
            touch_ps = psTch.tile([1, 16], f32, tag="touch", name="touch_ps")
            tcnt = [0]

            def touch(lhs_col, rhs_col):
                i = tcnt[0]
                tcnt[0] += 1
                nc.tensor.matmul(touch_ps[0:1, i:i + 1], lhs_col, rhs_col,
                                 start=True, stop=True)

            touch(ident[:, 0:1], ident[:, 0:1])
            touch(wv_sb[:, 0:1], wv_sb[:, 0:1])
            touch(wq_sb[:, 0:1], wq_sb[:, 0:1])
            touch(wk_sb[:, 0:1], wk_sb[:, 0:1])
            touch(ow_sb[:, 0:1], ow_sb[:, 0:1])

            xq_sb = xin.tile([128, 4096], f32, tag="xq")
            xk_sb = xin.tile([128, 4096], f32, tag="xk")
            xv_sb = xin.tile([128, 4096], f32, tag="xv")
            nc.sync.dma_start(out=xq_sb.rearrange("p (c q) -> p c q", c=4),
                              in_=xq_d[:].rearrange("(c p) q -> p c q", p=128))
            nc.sync.dma_start(out=xk_sb.rearrange("p (c q) -> p c q", c=4),
                              in_=xk_d[:].rearrange("(c p) q -> p c q", p=128))
            nc.sync.dma_start(out=xv_sb.rearrange("p (c q) -> p c q", c=4),
                              in_=xv_d[:].rearrange("(c p) q -> p c q", p=128))

            # v: out[k, d_all] per k-chunk kc8, interleaved 65-col head blocks
            for kc8 in range(8):
                ps = psA.tile([128, 512], f32, tag="psA")
                for kc in range(4):
                    nc.tensor.matmul(
                        ps[:, 0:256],
                        xv_sb[:, kc * 1024 + kc8 * 128: kc * 1024 + kc8 * 128 + 128],
                        wv_sb[:, kc * 256: kc * 256 + 256],
                        start=(kc == 0), stop=(kc == 3),
                    )
                v_view = v_sb[:, kc8 * 260: kc8 * 260 + 260].rearrange(
                    "p (h e) -> p h e", h=4)
                nc.vector.tensor_add(
                    v_view[:, :, 0:64],
                    ps[:, 0:256].rearrange("p (h e) -> p h e", h=4),
                    bv_sb.rearrange("p (h e) -> p h e", h=4))
                nc.vector.memset(v_view[:, :, 64:65], 1.0)

            # qT/kT: out[m*128+p, n] = sum_f w[f, m*128+p] * x[f, n]
            for x_sb, w_sb, b_sb, dst in ((xq_sb, wq_sb, bq_sb, qT_sb),
                                          (xk_sb, wk_sb, bk_sb, kT_sb)):
                for mi in range(2):
                    for ni in range(2):
                        ps = psA.tile([128, 512], f32, tag="psA")
                        for kc in range(4):
                            nc.tensor.matmul(
                                ps,
                                w_sb[:, kc * 256 + mi * 128: kc * 256 + mi * 128 + 128],
                                x_sb[:, kc * 1024 + ni * 512: kc * 1024 + ni * 512 + 512],
                                start=(kc == 0), stop=(kc == 3),
                            )
                        nc.vector.tensor_scalar_add(
                            dst[:, mi * 1024 + ni * 512: mi * 1024 + ni * 512 + 512],
                            ps, b_sb[:, mi: mi + 1])

            # absorb the DVE ticks of every v-region write (evac + ones
            # memset) so the first AV matmul of each k-chunk carries only
            # its ACT wait.
            for kc8 in range(8):
                touch(v_sb[:, kc8 * 260 + 64: kc8 * 260 + 65],
                      v_sb[:, kc8 * 260: kc8 * 260 + 1])

        # ---- stage C: attention ----
        with tc.tile_pool(name="kern", bufs=4) as kpool, \
             tc.tile_pool(name="mask", bufs=4) as mpool, \
             tc.tile_pool(name="esb", bufs=3) as epool, \
             tc.tile_pool(name="rec", bufs=4) as rpool, \
             tc.tile_pool(name="psS", bufs=2, space="PSUM") as psS, \
             tc.tile_pool(name="psO", bufs=6, space="PSUM") as psO:
            for h in range(4):
                p0 = 64 * (h % 2)          # partition offset of head h in qT/kT
                c0 = (h // 2) * 1024       # column offset of head h's m-chunk
                for qc in range(2):
                    o_tiles = [psO.tile([128, 65], f32, tag="opsum", name=f"o_{h}_{qc}_{qi}")
                               for qi in range(4)]
                    for kc in range(8):
                        kern_t = kpool.tile([128, 512], f32, tag="kern")
                        mask_t = mpool.tile([128, 512], u8, tag="mask")
                        nc.sync.dma_start(
                            out=kern_t,
                            in_=kern_d[h, kc * 128: kc * 128 + 128,
                                       qc * 512: qc * 512 + 512])
                        nc.sync.dma_start(
                            out=mask_t,
                            in_=mask_d[h, kc * 128: kc * 128 + 128,
                                       qc * 512: qc * 512 + 512])
                        nc.vector.copy_predicated(kern_t, mask_t, neginf)

                        s_ps = psS.tile([128, 512], f32, tag="spsum")
                        nc.tensor.matmul(
                            s_ps,
                            kT_sb[p0: p0 + 64, c0 + kc * 128: c0 + kc * 128 + 128],
                            qT_sb[p0: p0 + 64, c0 + qc * 512: c0 + qc * 512 + 512],
                            start=True, stop=False)
                        nc.tensor.matmul(s_ps, ident, kern_t, start=False, stop=True)

                        e_sb = epool.tile([128, 512], f32, tag="esb")
                        nc.scalar.activation(e_sb, s_ps, EXP)

                        for qi in range(4):
                            nc.tensor.matmul(
                                o_tiles[qi],
                                e_sb[:, qi * 128: qi * 128 + 128],
                                v_sb[:, kc * 260 + h * 65: kc * 260 + h * 65 + 65],
                                start=(kc == 0), stop=(kc == 7))

                    for qi in range(4):
                        qq = qc * 4 + qi
                        rec = rpool.tile([128, 1], f32, tag="rec")
                        nc.vector.reciprocal(rec, o_tiles[qi][:, 64:65])
                        nc.vector.tensor_scalar_mul(
                            on_sb[:, qq * 256 + h * 64: qq * 256 + h * 64 + 64],
                            o_tiles[qi][:, 0:64], rec)

        # ---- stage D: transpose + output projection ----
        with tc.tile_pool(name="outp", bufs=3) as outp, \
             tc.tile_pool(name="psT", bufs=2, space="PSUM") as psT, \
             tc.tile_pool(name="psR", bufs=2, space="PSUM") as psR:
            for qq in range(8):
                for j in range(2):
                    tp = psT.tile([128, 128], f32, tag="tp")
                    nc.tensor.transpose(
                        tp, on_sb[:, qq * 256 + j * 128: qq * 256 + j * 128 + 128],
                        ident)
                    nc.vector.tensor_copy(
                        ont_sb[:, j * 1024 + qq * 128: j * 1024 + qq * 128 + 128], tp)
                raw = psR.tile([128, 512], f32, tag="raw")
                for j in range(2):
                    nc.tensor.matmul(
                        raw,
                        ont_sb[:, j * 1024 + qq * 128: j * 1024 + qq * 128 + 128],
                        ow_sb[:, j * 512: j * 512 + 512],
                        start=(j == 0), stop=(j == 1))
                osb = outp.tile([128, 512], f32, tag="osb")
                nc.vector.tensor_copy(osb, raw)
                nc.sync.dma_start(out=out_d[qq * 128: qq * 128 + 128, :], in_=osb)

    nc.compile()
    return nc


def _get_program():
    global _PROGRAM
    if _PROGRAM is None:
        _PROGRAM = _build_program()
    return _PROGRAM


def build_in_maps(query, key_, value, attn_mask, kernel,
                  in_proj_weight, in_proj_bias, out_w):
    query = np.asarray(query, np.float32)
    key_ = np.asarray(key_, np.float32)
    value = np.asarray(value, np.float32)
    attn_mask = np.asarray(attn_mask)
    kernel = np.asarray(kernel, np.float32)
    in_proj_weight = np.asarray(in_proj_weight, np.float32)
    in_proj_bias = np.asarray(in_proj_bias, np.float32)
    out_w = np.asarray(out_w, np.float32)

    in_maps = []
    for c in range(NCORES):
        b = c // 2
        r0 = 256 * (c % 2)
        in_maps.append(dict(
            xq=np.ascontiguousarray(query[b].T),
            xk=np.ascontiguousarray(key_[b].T),
            xv=np.ascontiguousarray(value[b].T),
            wq=np.ascontiguousarray((in_proj_weight[r0:r0 + 256] * SCALE).T),
            wk=np.ascontiguousarray(in_proj_weight[512 + r0:512 + r0 + 256].T),
            wv=np.ascontiguousarray(in_proj_weight[1024 + r0:1024 + r0 + 256].T),
            bq=np.ascontiguousarray((in_proj_bias[r0:r0 + 256] * SCALE)
                                    .reshape(2, 128).T),
            bk=np.ascontiguousarray(in_proj_bias[512 + r0:512 + r0 + 256]
                                    .reshape(2, 128).T),
            bv=np.ascontiguousarray(in_proj_bias[1024 + r0:1024 + r0 + 256]
                                    .reshape(1, 256)),
            ow=np.ascontiguousarray(out_w[:, r0:r0 + 256].T),
            kern=np.ascontiguousarray(kernel[4 * c:4 * c + 4]),
            maskT=np.ascontiguousarray(
                attn_mask[4 * c:4 * c + 4].transpose(0, 2, 1)).astype(np.uint8),
        ))
    return in_maps


def kernel(query, key_, value, attn_mask, key_padding_mask, kernel,
           in_proj_weight, in_proj_bias, out_w, out_b):
    from concourse.bass_utils import run_bass_kernel_spmd

    out_b = np.asarray(out_b, np.float32)
    nc = _get_program()
    in_maps = build_in_maps(query, key_, value, attn_mask, kernel,
                            in_proj_weight, in_proj_bias, out_w)
    results = run_bass_kernel_spmd(nc, in_maps, list(range(NCORES))).results

    out = np.empty((B, L, HD), np.float32)
    for b in range(B):
        out[b] = results[2 * b]["partial"] + results[2 * b + 1]["partial"]
    out += out_b
    return out
